# revision 45
# baseline (speedup 1.0000x reference)
"""Trainium2 Bass kernel for nn_ExampleModel_1116691497724 (moe_routing).

Math: the reference returns log_softmax_T( sum_D(moe_out) ), and sum_D
collapses the expert FFN to a dot product:
    sum_d (h @ W2[e] + b2[e]) = h . w2sum[e] + sum(b2[e]),  w2sum[e] = W2[e] @ 1
    (x @ W1[e] + b1[e]) . w2sum[e] = x . v[e] + c[e]
with v[e] = W1[e] @ w2sum[e]  (a [D] vector) and scalar
c[e] = b1[e].w2sum[e] + sum(b2[e]).  Then per token:
    s_e = x . v[e] + c[e],  logits = x @ Wg
    moe_sum = max(softmax(logits)) * s_argmax(logits)
    out = log_softmax over tokens (per batch row) of moe_sum.

Distribution over 8 cores, two launches (measured here: a 16KB 8-core ncfw
AllReduce costs ~78us on this runtime vs ~16us for a whole extra trivial
launch, so the 16KB cross-core combine happens on the host between launches;
the host does only that partial sum plus layout shuffles, all real math
stays on device):
  launch A (expert-parallel over H): core c reduces W2[:, 128c:128c+128, :]
    (bf16, host-cast: the v path tolerates ~2e-3 rel err; the argmax/logits
    path does not and stays f32) and computes partial v from the matching W1
    columns -> outputs [v0 | v1 | c0 c1] partials (16KB); host sums.
  launch B (token-parallel): core c owns batch row c%4 (512 tokens); one
    f32r M=3 stream with host-prepped stationary [wg0-wg1, v0-v1, v1]
    yields delta, s0-s1, s1 per token directly (1 cycle/row, verified
    exact enough that no argmax tie flips vs the fp32 reference on this
    input); gate/select vectorized across all 4 token groups via
    gate = 1/(1+exp(-|delta|)); row log_softmax via PE transposes.

Perf notes baked in from traces:
  - x rides the two HWDGE rings (ACT + SP), chunks emitted round-robin so
    the single in-order PSUM accumulation chain consumes them in roughly
    arrival order; the ACT ring measures faster than the SP ring here, so
    byte splits favor it, and the final chunks are single d-blocks so only
    one matmul trails the last byte.  (A third SWDGE x path measured the
    same mean but much higher variance - SWDGE descgen starts late and a
    mid-chain stall blocks all later PE-FIFO matmuls.)
  - launch A computes v TRANSPOSED (stationary = 128-col W1 chunks bf16,
    moving = w2sum, N=1): all of v lands in one [128, 32] PSUM tile ->
    one wide DVE copy instead of serial single-partition copies.
  - the stationary m4 ([wg0-wg1, v0-v1, v1] per d-block), gating consts and
    the transpose identity are host-prepared and DMA'd: no on-device
    setup transposes and no ACT-table churn (make_identity loads tables).
  - the ACT engine reloads its table on every function switch (1.28us), so
    the flow uses exactly one switch-free Exp sequence; the final log() is
    computed on the DVE via exponent/mantissa bit extraction (max err 0.06
    abs vs the 3.5 abs output tolerance) -- no Ln table load at all.
  - dummy matmuls warm the PE (HAM un-throttles after ~3.4us busy) before
    the real stream arrives; an explicit scheduler dep keeps them ordered
    BEFORE the stream in the PE FIFO.
"""

import sys

import numpy as np

for _p in ("/opt/trn_rl_repo",):
    if _p not in sys.path:
        sys.path.append(_p)

import concourse.bass as bass  # noqa: E402
import concourse.mybir as mybir  # noqa: E402
import concourse.tile as tile  # noqa: E402
from concourse import bacc, bass_utils  # noqa: E402
from concourse.tile_rust import add_dep_helper  # noqa: E402

# Problem shape (hardcoded per spec).
B, T, D, H, E = 4, 512, 2048, 1024, 2
P = 128
NCORES = 8
TB = T  # tokens per core = one batch row
NB = D // P  # 16 d-blocks
HC = H // NCORES  # 128 h-chunk per expert per core
NG = TB // P  # 4 token groups per core
DC = D // NCORES  # 256 b2 columns per core
F32 = mybir.dt.float32
F32R = mybir.dt.float32r
BF16 = mybir.dt.bfloat16
AX = mybir.AxisListType
AF = mybir.ActivationFunctionType
ALU = mybir.AluOpType

VPART = 2 * D + 2  # launch A output: v0 | v1 | c0 c1


def emit_phase_a(nc, tc, io):
    """w2sum + partial v for this core's H-chunk -> vout [128, 32] + cout."""
    w1c, w2c, b1c, b2c = io["w1c"], io["w2c"], io["b1c"], io["b2c"]
    vout, cout = io["vout"], io["cout"]
    with (
        tc.tile_pool(name="main", bufs=1) as pool,
        tc.tile_pool(name="psum", bufs=1, space="PSUM") as psum,
    ):
        # tiny inputs on the SWDGE ring so they never head-of-line block the
        # big HWDGE streams; b1 arrives already partition-major [128, E]
        b1p = pool.tile([P, E], F32)
        nc.gpsimd.dma_start(b1p[:], b1c)
        b2_sb = pool.tile([1, E * DC], F32)
        nc.gpsimd.dma_start(b2_sb[:], b2c)

        # W2 first (gates the reduce -> w2sum -> everything), then W1, both
        # bf16 [128, E*D].  The ACT ring measures ~2x the SP ring's rate, so
        # it carries ~2/3 of the bytes.  Chunks sized so reduces pipeline.
        w2_sb = pool.tile([P, E * D], BF16)
        w1r = pool.tile([P, E * D], BF16)
        # col ranges per ring: scalar gets [0, 2560), sync gets [2560, 4096)
        SPL = 2560  # multiple of 512; ~5/8 to the fast ring
        for a, b, ring in ((0, SPL, nc.scalar), (SPL, E * D, nc.sync)):
            ring.dma_start(w2_sb[:, a:b], w2c[:, a:b])
        for a, b, ring in ((0, SPL, nc.scalar), (SPL, E * D, nc.sync)):
            ring.dma_start(w1r[:, a:b], w1c[:, a:b])

        # w2sum: reduce each W2 piece as it lands (bf16 input -> 2x DVE rate),
        # then combine per expert.  piece boundaries = DMA chunk boundaries
        # intersected with expert boundaries.
        bounds = sorted({0, SPL, E * D, D, 2 * D})
        pieces = [(a, b) for a, b in zip(bounds[:-1], bounds[1:])]
        w2h = pool.tile([P, len(pieces)], F32)
        for i, (a, b) in enumerate(pieces):
            nc.vector.reduce_sum(w2h[:, i : i + 1], w2_sb[:, a:b], axis=AX.X)
        w2s = pool.tile([P, E], F32)
        for e in range(E):
            idxs = [i for i, (a, b) in enumerate(pieces) if a >= e * D and b <= (e + 1) * D]
            if len(idxs) == 1:
                nc.vector.tensor_copy(w2s[:, e : e + 1], w2h[:, idxs[0] : idxs[0] + 1])
            else:
                acc = w2h[:, idxs[0] : idxs[0] + 1]
                for i in idxs[1:]:
                    nc.vector.tensor_add(w2s[:, e : e + 1], acc, w2h[:, i : i + 1])
                    acc = w2s[:, e : e + 1]
        w2s_b = pool.tile([P, E], BF16)
        nc.vector.tensor_copy(w2s_b[:], w2s[:])
        b2s = pool.tile([1, E], F32)
        for e in range(E):
            nc.vector.reduce_sum(
                b2s[0:1, e : e + 1], b2_sb[0:1, e * DC : (e + 1) * DC], axis=AX.X
            )

        # v transposed: stationary = 128-col W1 chunk (bf16 -> fast weight
        # load), moving = w2sum column (N=1).  Every chunk lands in its own
        # column of ONE [128, 32] PSUM tile -> a single wide copy replaces 8
        # serial single-partition copies.  vout[p, k] = v[k//16][(k%16)*128+p]
        b1dot = psum.tile([1, E], F32)
        v128 = psum.tile([P, E * NB], F32)
        for k in range(E * NB):
            e = k * P // D
            nc.tensor.matmul(
                v128[:, k : k + 1],
                w1r[:, k * P : (k + 1) * P],
                w2s_b[:, e : e + 1],
                start=True,
                stop=True,
            )
        vsb = pool.tile([P, E * NB], F32)
        nc.vector.tensor_copy(vsb[:], v128[:])
        nc.sync.dma_start(vout[:], vsb[:])
        cpay = pool.tile([1, E], F32)
        for e in range(E):
            nc.tensor.matmul(
                b1dot[0:1, e : e + 1],
                w2s[:, e : e + 1],
                b1p[:, e : e + 1],
                start=True,
                stop=True,
            )
            nc.vector.tensor_add(
                cpay[0:1, e : e + 1], b1dot[0:1, e : e + 1], b2s[0:1, e : e + 1]
            )
        nc.scalar.dma_start(cout[:], cpay[:])


# x chunks in emission order (engine, d-blocks): round-robin across the
# three DMA paths so the single in-order PSUM accumulation chain consumes
# chunks in roughly arrival order; bytes split ~per measured path rate.
XCHUNKS = [
    ("scalar", (0, 1)),
    ("sync", (2, 3)),
    ("scalar", (4, 5)),
    ("sync", (6, 7)),
    ("scalar", (8, 9)),
    ("sync", (10, 11)),
    ("scalar", (12, 13)),
    ("sync", (15,)),
    ("scalar", (14,)),
]


def emit_phase_b(nc, tc, io):
    """One f32r M=4 stream (logits+s), vectorized gating, row log_softmax."""
    xc, m4h, identh, consth, out = io["xc"], io["m4h"], io["identh"], io["consth"], io["out"]
    with (
        tc.tile_pool(name="main", bufs=1) as pool,
        tc.tile_pool(name="psum", bufs=1, space="PSUM") as psum,
    ):
        # small setup tensors ride the SWDGE ring ahead of its x share; the
        # identity (needed only for the late transposes) rides after it.
        # consth = host-built [cb16 (NG*4) | zz (NG) | one1 (1)] per partition
        m4 = pool.tile([P, NB, 3], F32R)
        nc.gpsimd.dma_start(m4[:], m4h)
        consts = pool.tile([P, NG * 3 + NG + 1], F32)
        nc.gpsimd.dma_start(consts[:], consth)
        cb16 = consts[:, 0 : NG * 3].rearrange("p (g k) -> p g k", k=3)
        zz = consts[:, NG * 3 : NG * 3 + NG]
        one1 = consts[:, NG * 3 + NG : NG * 3 + NG + 1]

        # x [128, NB*TB] f32r (d = n*128 + p): three DMA paths (the two HWDGE
        # rings + SWDGE), chunks emitted round-robin so the single in-order
        # PSUM accumulation chain consumes them in roughly arrival order
        x_sb = pool.tile([P, NB * TB], F32R)
        engines = {"scalar": nc.scalar, "sync": nc.sync, "gpsimd": nc.gpsimd}
        for path, blks in XCHUNKS:
            a, b = blks[0] * TB, (blks[-1] + 1) * TB
            engines[path].dma_start(x_sb[:, a:b], xc[:, a:b])
        ident = pool.tile([P, P], F32)
        nc.gpsimd.dma_start(ident[:], identh)

        # preload the Exp ACT table; reading from the first x chunk makes it
        # wait until that DMA lands, so the table load never delays the ACT
        # ring's own DMA triggers.  Ln loads late, after the last Exp use.
        wz = pool.tile([1, 2], F32)
        warm = pool.tile([1, 2], F32)
        nc.vector.tensor_copy(warm[:], x_sb[0:1, 0:2])
        nc.scalar.activation(wz[:], warm[:], AF.Exp)

        # PE warmup: HAM un-throttles (1.2 -> 2.4 GHz) after ~3.4us of
        # sustained activity.  The explicit dep on the first stream matmul
        # keeps the dummies ORDERED BEFORE the stream in the PE FIFO (the
        # scheduler would otherwise interleave them into it).
        wmm = pool.tile([P, 512], BF16)
        nc.vector.memset(wmm[:], 0.0)
        wms = pool.tile([P, 1], BF16)
        nc.vector.memset(wms[:], 0.0)
        last_dummy = None
        for i in range(8):
            wmo = psum.tile([1, 512], F32, name="wmo", tag="wmo", bufs=2)
            last_dummy = nc.tensor.matmul(wmo[:], wms[:], wmm[:], start=True, stop=True)

        # single accumulation chain in chunk-emission order
        ps4 = psum.tile([3, TB], F32)
        nchunks = len(XCHUNKS)
        first_mm = None
        for ci, (path, blks) in enumerate(XCHUNKS):
            for j, n in enumerate(blks):
                mm = nc.tensor.matmul(
                    ps4[:],
                    m4[:, n, :],
                    x_sb[:, n * TB : (n + 1) * TB],
                    start=(ci == 0 and j == 0),
                    stop=(ci == nchunks - 1 and j == len(blks) - 1),
                )
                if first_mm is None:
                    first_mm = mm
        add_dep_helper(first_mm.ins, last_dummy.ins, sync=False, reason="PE warmup before stream")
        sbl = pool.tile([3, TB], F32)
        for g in range(NG):
            nc.vector.tensor_copy(
                sbl[:, g * P : (g + 1) * P], ps4[:, g * P : (g + 1) * P]
            )

        # gating, vectorized across all 4 token groups: t16[:, g, :] holds
        # [delta=l0-l1, sd=s0-s1, s1] for tokens g*128..g*128+127 (the
        # differences come straight off the stream via host-prepped
        # [wg0-wg1, v0-v1, v1] stationary columns)
        t16_ps = psum.tile([P, NG, 3], F32)
        for g in range(NG):
            nc.tensor.transpose(
                t16_ps[:, g, :], sbl[0:3, g * P : (g + 1) * P], ident[0:3, 0:3]
            )
        t16 = pool.tile([P, NG, 3], F32)
        nc.vector.tensor_add(t16[:], t16_ps[:], cb16[:])
        ndl = pool.tile([P, NG], F32)
        nc.vector.tensor_sub(ndl[:, :, None], zz[:, :, None], t16[:, :, 0:1])
        mneg = pool.tile([P, NG], F32)
        nc.vector.tensor_tensor(
            mneg[:, :, None], t16[:, :, 0:1], ndl[:, :, None], op=ALU.min
        )  # -|delta|
        eneg = pool.tile([P, NG], F32)
        nc.scalar.activation(eneg[:], mneg[:], AF.Exp)
        den = pool.tile([P, NG], F32)
        nc.vector.tensor_scalar_add(den[:], eneg[:], one1[:])
        gate = pool.tile([P, NG], F32)
        nc.vector.reciprocal(gate[:], den[:])  # = max softmax prob
        mask = pool.tile([P, NG], F32)
        nc.vector.tensor_tensor(
            mask[:, :, None], t16[:, :, 0:1], zz[:, :, None], op=ALU.is_ge
        )
        msd = pool.tile([P, NG], F32)
        nc.vector.tensor_mul(msd[:, :, None], mask[:, :, None], t16[:, :, 1:2])
        ssel = pool.tile([P, NG], F32)
        nc.vector.tensor_add(ssel[:, :, None], msd[:, :, None], t16[:, :, 2:3])
        moe_sb = pool.tile([P, NG], F32)
        nc.vector.tensor_mul(moe_sb[:], gate[:], ssel[:])

        # row log_softmax over all 512 tokens, via PE transposes; the max
        # reduction runs on the [128, 4] layout in parallel with the big
        # transpose (128 lanes instead of 4)
        m128 = pool.tile([P, 1], F32)
        nc.vector.reduce_max(m128[:], moe_sb[:], axis=AX.X)
        tp4 = psum.tile([NG, P], F32)
        nc.tensor.transpose(tp4[:], moe_sb[:], ident[:])
        mtp = psum.tile([1, P], F32, name="mtp", tag="t1", bufs=1)
        nc.tensor.transpose(mtp[:], m128[:], ident[:])
        sb4t = pool.tile([NG, P], F32)
        nc.vector.tensor_copy(sb4t[:], tp4[:])
        negm2 = pool.tile([1, 1], F32)
        nc.vector.reduce_max(negm2[:], mtp[:], axis=AX.X, negate=True)
        negm4 = pool.tile([NG, 1], F32)
        nc.gpsimd.partition_broadcast(negm4[:], negm2[:])
        e4 = pool.tile([NG, P], F32)
        s4 = pool.tile([NG, 1], F32)
        nc.scalar.activation(e4[:], sb4t[:], AF.Exp, bias=negm4[:], accum_out=s4[:])
        s1p = psum.tile([1, NG], F32, name="s1p", tag="t1", bufs=1)
        nc.tensor.transpose(s1p[:], s4[:], ident[0:NG, 0:NG])
        ssum = pool.tile([1, 1], F32)
        nc.vector.reduce_sum(ssum[:], s1p[:], axis=AX.X)
        # log(ssum) via exponent/mantissa bit extraction on the DVE: avoids
        # the Exp->Ln ACT table switch (a 1.28us table load on the critical
        # path).  log(2^k * m) ~= (k + (m-1)) * ln2, max err 0.06 abs --
        # tiny next to the 2e-2 * 175 = 3.5 abs output tolerance.
        I32 = mybir.dt.int32
        si = ssum[0:1, 0:1].bitcast(I32)
        sr = pool.tile([1, 1], I32)
        nc.vector.tensor_scalar(sr[:], si, 23, None, op0=ALU.arith_shift_right)
        kf = pool.tile([1, 1], F32)
        nc.vector.tensor_copy(kf[:], sr[:])
        mi = pool.tile([1, 1], I32)
        nc.vector.tensor_scalar(
            mi[:], si, 0x7FFFFF, 0x3F800000, op0=ALU.bitwise_and, op1=ALU.bitwise_or
        )
        tsum = pool.tile([1, 1], F32)
        nc.vector.tensor_add(tsum[:], kf[:], mi[0:1, 0:1].bitcast(F32))
        logs = pool.tile([1, 1], F32)
        nc.vector.tensor_scalar(
            logs[:], tsum[:], -128.0, 0.6931471805599453, op0=ALU.add, op1=ALU.mult
        )
        shift = pool.tile([1, 1], F32)
        nc.vector.tensor_sub(shift[:], negm2[:], logs[:])
        shift4 = pool.tile([NG, 1], F32)
        nc.gpsimd.partition_broadcast(shift4[:], shift[:])
        res4 = pool.tile([NG, P], F32)
        nc.vector.tensor_scalar_add(res4[:], sb4t[:], shift4[:])
        nc.sync.dma_start(out.rearrange("x (g p) -> g (x p)", p=P), res4[:])


_CACHED = {}


def build_program(which):
    if which in _CACHED:
        return _CACHED[which]
    nc = bacc.Bacc(
        "TRN2",
        target_bir_lowering=False,
        debug=False,
        enable_asserts=False,
        num_devices=NCORES,
    )
    if which == "a":
        io = {
            "w1c": nc.dram_tensor("w1c", [P, E * D], BF16, kind="ExternalInput").ap(),
            "w2c": nc.dram_tensor("w2c", [P, E * D], BF16, kind="ExternalInput").ap(),
            "b1c": nc.dram_tensor("b1c", [P, E], F32, kind="ExternalInput").ap(),
            "b2c": nc.dram_tensor("b2c", [1, E * DC], F32, kind="ExternalInput").ap(),
            "vout": nc.dram_tensor("vout", [P, E * NB], F32, kind="ExternalOutput").ap(),
            "cout": nc.dram_tensor("cout", [1, E], F32, kind="ExternalOutput").ap(),
        }
        emit = emit_phase_a
    else:
        io = {
            "xc": nc.dram_tensor("xc", [P, NB * TB], F32R, kind="ExternalInput").ap(),
            "m4h": nc.dram_tensor("m4h", [P, NB, 3], F32R, kind="ExternalInput").ap(),
            "identh": nc.dram_tensor("identh", [P, P], F32, kind="ExternalInput").ap(),
            "consth": nc.dram_tensor(
                "consth", [P, NG * 3 + NG + 1], F32, kind="ExternalInput"
            ).ap(),
            "out": nc.dram_tensor("out", [1, TB], F32, kind="ExternalOutput").ap(),
        }
        emit = emit_phase_b
    with tile.TileContext(nc) as tc:
        emit(nc, tc, io)
    nc.compile()
    _CACHED[which] = nc
    return nc


def shard_inputs_a(Wg, W1, b1, W2, b2):
    import ml_dtypes

    W1 = np.asarray(W1, np.float32)
    b1 = np.asarray(b1, np.float32)
    W2 = np.asarray(W2, np.float32)
    b2 = np.asarray(b2, np.float32)
    in_maps = []
    for c in range(NCORES):
        hs, he = c * HC, (c + 1) * HC
        # w1c[p, e*D+d] = W1[e, d, hs+p];  w2c[p, e*D+d] = W2[e, hs+p, d]
        in_maps.append(
            {
                "w1c": np.ascontiguousarray(
                    W1[:, :, hs:he].transpose(2, 0, 1).reshape(HC, E * D)
                ).astype(ml_dtypes.bfloat16),
                "w2c": np.ascontiguousarray(
                    W2[:, hs:he, :].transpose(1, 0, 2).reshape(HC, E * D)
                ).astype(ml_dtypes.bfloat16),
                "b1c": np.ascontiguousarray(b1[:, hs:he].T),  # [128, E]
                "b2c": np.ascontiguousarray(
                    b2[:, c * DC : (c + 1) * DC].reshape(1, E * DC)
                ),
            }
        )
    return in_maps


def shard_inputs_b(x, Wg, vpart_sum):
    x = np.asarray(x, np.float32)
    Wg = np.asarray(Wg, np.float32)
    vp = np.asarray(vpart_sum, np.float32).reshape(-1)
    # m4h[p, n, :] = [wg0-wg1, v0-v1, v1] at d = n*128+p
    m4h = np.empty((P, NB, 3), np.float32)
    wg_pn = Wg.reshape(NB, P, E).transpose(1, 0, 2)  # [p, n, e]
    m4h[:, :, 0] = wg_pn[:, :, 0] - wg_pn[:, :, 1]
    v0 = vp[0:D].reshape(NB, P).T
    v1 = vp[D : 2 * D].reshape(NB, P).T
    m4h[:, :, 1] = v0 - v1
    m4h[:, :, 2] = v1
    m4h = np.ascontiguousarray(m4h)
    identh = np.eye(P, dtype=np.float32)
    # consth = [cb (NG*3: [0, c0-c1, c1] per group) | zz (NG) | one (1)]
    consth = np.zeros((P, NG * 3 + NG + 1), np.float32)
    cs = vp[2 * D : 2 * D + E]
    for g in range(NG):
        consth[:, g * 3 + 1] = cs[0] - cs[1]
        consth[:, g * 3 + 2] = cs[1]
    consth[:, NG * 3 + NG] = 1.0
    in_maps = []
    for c in range(NCORES):
        row = c % B
        # xc[p, n*TB + t] = x[row, t, n*128 + p]
        xr = np.ascontiguousarray(
            x[row].reshape(TB, NB, P).transpose(2, 1, 0).reshape(P, NB * TB)
        )
        in_maps.append({"xc": xr, "m4h": m4h, "identh": identh, "consth": consth})
    return in_maps


def run_a(in_maps, **kwargs):
    return bass_utils.run_bass_kernel_spmd(
        build_program("a"), in_maps, core_ids=list(range(NCORES)), **kwargs
    )


def run_b(in_maps, **kwargs):
    return bass_utils.run_bass_kernel_spmd(
        build_program("b"), in_maps, core_ids=list(range(NCORES)), **kwargs
    )


def combine_vparts(res_a):
    """Sum the 8 per-core [128, 32] v partials + [1, 2] c partials into the
    flat [1, 2D+2] vpart layout (v0 | v1 | c0 c1) launch B consumes.

    vout[p, k] = v_partial[k*128//D][(k%NB)*128 + p]."""
    v128 = np.sum([res_a.results[c]["vout"] for c in range(NCORES)], axis=0)
    cs = np.sum([res_a.results[c]["cout"] for c in range(NCORES)], axis=0)
    v = v128.T.reshape(E, NB, P).reshape(E * D)  # [e, j, p] -> flat
    vpart = np.concatenate([v, cs.reshape(-1)]).reshape(1, VPART)
    return np.ascontiguousarray(vpart, np.float32)


def kernel(x, Wg, W1, b1, W2, b2):
    res_a = run_a(shard_inputs_a(Wg, W1, b1, W2, b2))
    # cross-core combine: sum of the 8 per-core partials (the gather/reshard
    # step between the two launches; 16KB, no model math beyond the reduction)
    vpart = combine_vparts(res_a)
    res_b = run_b(shard_inputs_b(x, Wg, vpart))
    return np.concatenate([res_b.results[b]["out"] for b in range(B)], axis=0)


# revision 46
# speedup vs baseline: 1.0176x; 1.0176x over previous
"""Trainium2 Bass kernel for nn_ExampleModel_1116691497724 (moe_routing).

Math: the reference returns log_softmax_T( sum_D(moe_out) ), and sum_D
collapses the expert FFN to a dot product:
    sum_d (h @ W2[e] + b2[e]) = h . w2sum[e] + sum(b2[e]),  w2sum[e] = W2[e] @ 1
    (x @ W1[e] + b1[e]) . w2sum[e] = x . v[e] + c[e]
with v[e] = W1[e] @ w2sum[e]  (a [D] vector) and scalar
c[e] = b1[e].w2sum[e] + sum(b2[e]).  Then per token:
    s_e = x . v[e] + c[e],  logits = x @ Wg
    moe_sum = max(softmax(logits)) * s_argmax(logits)
    out = log_softmax over tokens (per batch row) of moe_sum.

Distribution over 8 cores, two launches (measured here: a 16KB 8-core ncfw
AllReduce costs ~78us on this runtime vs ~16us for a whole extra trivial
launch, so the 16KB cross-core combine happens on the host between launches;
the host does only that partial sum plus layout shuffles, all real math
stays on device):
  launch A (expert-parallel over H): core c reduces W2[:, 128c:128c+128, :]
    (bf16, host-cast: the v path tolerates ~2e-3 rel err; the argmax/logits
    path does not and stays f32) and computes partial v from the matching W1
    columns -> outputs [v0 | v1 | c0 c1] partials (16KB); host sums.
  launch B (token-parallel): core c owns batch row c%4 (512 tokens); one
    f32r M=3 stream with host-prepped stationary [wg0-wg1, v0-v1, v1]
    yields delta, s0-s1, s1 per token directly (1 cycle/row, verified
    exact enough that no argmax tie flips vs the fp32 reference on this
    input); gate/select vectorized across all 4 token groups via
    gate = 1/(1+exp(-|delta|)); row log_softmax via PE transposes.

Perf notes baked in from traces:
  - x rides the two HWDGE rings (ACT + SP), chunks emitted round-robin so
    the single in-order PSUM accumulation chain consumes them in roughly
    arrival order; the ACT ring measures faster than the SP ring here, so
    byte splits favor it, and the final chunks are single d-blocks so only
    one matmul trails the last byte.  (A third SWDGE x path measured the
    same mean but much higher variance - SWDGE descgen starts late and a
    mid-chain stall blocks all later PE-FIFO matmuls.)
  - launch A computes v TRANSPOSED (stationary = 128-col W1 chunks bf16,
    moving = w2sum, N=1): all of v lands in one [128, 32] PSUM tile ->
    one wide DVE copy instead of serial single-partition copies.
  - the stationary m4 ([wg0-wg1, v0-v1, v1] per d-block), gating consts and
    the transpose identity are host-prepared and DMA'd: no on-device
    setup transposes and no ACT-table churn (make_identity loads tables).
  - the ACT engine reloads its table on every function switch (1.28us), so
    the flow uses exactly one switch-free Exp sequence; the final log() is
    computed on the DVE via exponent/mantissa bit extraction (max err 0.06
    abs vs the 3.5 abs output tolerance) -- no Ln table load at all.
  - dummy matmuls warm the PE (HAM un-throttles after ~3.4us busy) before
    the real stream arrives; an explicit scheduler dep keeps them ordered
    BEFORE the stream in the PE FIFO.
"""

import sys

import numpy as np

for _p in ("/opt/trn_rl_repo",):
    if _p not in sys.path:
        sys.path.append(_p)

import concourse.bass as bass  # noqa: E402
import concourse.mybir as mybir  # noqa: E402
import concourse.tile as tile  # noqa: E402
from concourse import bacc, bass_utils  # noqa: E402
from concourse.tile_rust import add_dep_helper  # noqa: E402

# Problem shape (hardcoded per spec).
B, T, D, H, E = 4, 512, 2048, 1024, 2
P = 128
NCORES = 8
TB = T  # tokens per core = one batch row
NB = D // P  # 16 d-blocks
HC = H // NCORES  # 128 h-chunk per expert per core
NG = TB // P  # 4 token groups per core
DC = D // NCORES  # 256 b2 columns per core
F32 = mybir.dt.float32
F32R = mybir.dt.float32r
BF16 = mybir.dt.bfloat16
AX = mybir.AxisListType
AF = mybir.ActivationFunctionType
ALU = mybir.AluOpType

VPART = 2 * D + 2  # launch A output: v0 | v1 | c0 c1


def emit_phase_a(nc, tc, io):
    """w2sum + partial v for this core's H-chunk -> vout [128, 32] + cout."""
    w1c, w2c, b1c, b2c = io["w1c"], io["w2c"], io["b1c"], io["b2c"]
    vout, cout = io["vout"], io["cout"]
    with (
        tc.tile_pool(name="main", bufs=1) as pool,
        tc.tile_pool(name="psum", bufs=1, space="PSUM") as psum,
    ):
        # tiny inputs on the SWDGE ring so they never head-of-line block the
        # big HWDGE streams; b1 arrives already partition-major [128, E]
        b1p = pool.tile([P, E], F32)
        nc.gpsimd.dma_start(b1p[:], b1c)
        b2_sb = pool.tile([1, E * DC], F32)
        nc.gpsimd.dma_start(b2_sb[:], b2c)

        # W2 first (gates the reduce -> w2sum -> everything), then W1, both
        # bf16 [128, E*D].  The ACT ring measures ~2x the SP ring's rate, so
        # it carries ~2/3 of the bytes.  Chunks sized so reduces pipeline.
        w2_sb = pool.tile([P, E * D], BF16)
        w1r = pool.tile([P, E * D], BF16)
        # col ranges per ring: scalar gets [0, 2560), sync gets [2560, 4096)
        SPL = 2560  # multiple of 512; ~5/8 to the fast ring
        for a, b, ring in ((0, SPL, nc.scalar), (SPL, E * D, nc.sync)):
            ring.dma_start(w2_sb[:, a:b], w2c[:, a:b])
        for a, b, ring in ((0, SPL, nc.scalar), (SPL, E * D, nc.sync)):
            ring.dma_start(w1r[:, a:b], w1c[:, a:b])

        # w2sum: reduce each W2 piece as it lands (bf16 input -> 2x DVE rate),
        # then combine per expert.  piece boundaries = DMA chunk boundaries
        # intersected with expert boundaries.
        bounds = sorted({0, SPL, E * D, D, 2 * D})
        pieces = [(a, b) for a, b in zip(bounds[:-1], bounds[1:])]
        w2h = pool.tile([P, len(pieces)], F32)
        for i, (a, b) in enumerate(pieces):
            nc.vector.reduce_sum(w2h[:, i : i + 1], w2_sb[:, a:b], axis=AX.X)
        w2s = pool.tile([P, E], F32)
        for e in range(E):
            idxs = [i for i, (a, b) in enumerate(pieces) if a >= e * D and b <= (e + 1) * D]
            if len(idxs) == 1:
                nc.vector.tensor_copy(w2s[:, e : e + 1], w2h[:, idxs[0] : idxs[0] + 1])
            else:
                acc = w2h[:, idxs[0] : idxs[0] + 1]
                for i in idxs[1:]:
                    nc.vector.tensor_add(w2s[:, e : e + 1], acc, w2h[:, i : i + 1])
                    acc = w2s[:, e : e + 1]
        w2s_b = pool.tile([P, E], BF16)
        nc.vector.tensor_copy(w2s_b[:], w2s[:])
        b2s = pool.tile([1, E], F32)
        for e in range(E):
            nc.vector.reduce_sum(
                b2s[0:1, e : e + 1], b2_sb[0:1, e * DC : (e + 1) * DC], axis=AX.X
            )

        # v transposed: stationary = 128-col W1 chunk (bf16 -> fast weight
        # load), moving = w2sum column (N=1).  Every chunk lands in its own
        # column of ONE [128, 32] PSUM tile -> a single wide copy replaces 8
        # serial single-partition copies.  vout[p, k] = v[k//16][(k%16)*128+p]
        b1dot = psum.tile([1, E], F32)
        v128 = psum.tile([P, E * NB], F32)
        for k in range(E * NB):
            e = k * P // D
            nc.tensor.matmul(
                v128[:, k : k + 1],
                w1r[:, k * P : (k + 1) * P],
                w2s_b[:, e : e + 1],
                start=True,
                stop=True,
            )
        vsb = pool.tile([P, E * NB], F32)
        nc.vector.tensor_copy(vsb[:], v128[:])
        nc.sync.dma_start(vout[:], vsb[:])
        cpay = pool.tile([1, E], F32)
        for e in range(E):
            nc.tensor.matmul(
                b1dot[0:1, e : e + 1],
                w2s[:, e : e + 1],
                b1p[:, e : e + 1],
                start=True,
                stop=True,
            )
            nc.vector.tensor_add(
                cpay[0:1, e : e + 1], b1dot[0:1, e : e + 1], b2s[0:1, e : e + 1]
            )
        nc.scalar.dma_start(cout[:], cpay[:])


# x chunks in emission order (engine, d-blocks): round-robin across the
# three DMA paths so the single in-order PSUM accumulation chain consumes
# chunks in roughly arrival order; bytes split ~per measured path rate.
XCHUNKS = [
    ("scalar", (0, 1)),
    ("sync", (2, 3)),
    ("scalar", (4, 5)),
    ("sync", (6, 7)),
    ("scalar", (8, 9)),
    ("sync", (10, 11)),
    ("scalar", (12, 13)),
    ("sync", (15,)),
    ("scalar", (14,)),
]


def emit_phase_b(nc, tc, io):
    """One f32r M=4 stream (logits+s), vectorized gating, row log_softmax."""
    xc, m4h, identh, consth, out = io["xc"], io["m4h"], io["identh"], io["consth"], io["out"]
    with (
        tc.tile_pool(name="main", bufs=1) as pool,
        tc.tile_pool(name="psum", bufs=1, space="PSUM") as psum,
    ):
        # small setup tensors ride the SWDGE ring ahead of its x share; the
        # identity (needed only for the late transposes) rides after it.
        # consth = host-built [cb16 (NG*4) | zz (NG) | one1 (1)] per partition
        m4 = pool.tile([P, NB, 3], F32R)
        nc.gpsimd.dma_start(m4[:], m4h)
        consts = pool.tile([P, NG * 3 + NG + 1], F32)
        nc.gpsimd.dma_start(consts[:], consth)
        cb16 = consts[:, 0 : NG * 3].rearrange("p (g k) -> p g k", k=3)
        zz = consts[:, NG * 3 : NG * 3 + NG]
        one1 = consts[:, NG * 3 + NG : NG * 3 + NG + 1]

        # x [128, NB*TB] f32r (d = n*128 + p): three DMA paths (the two HWDGE
        # rings + SWDGE), chunks emitted round-robin so the single in-order
        # PSUM accumulation chain consumes them in roughly arrival order
        x_sb = pool.tile([P, NB * TB], F32R)
        engines = {"scalar": nc.scalar, "sync": nc.sync, "gpsimd": nc.gpsimd}
        for path, blks in XCHUNKS:
            a, b = blks[0] * TB, (blks[-1] + 1) * TB
            engines[path].dma_start(x_sb[:, a:b], xc[:, a:b])
        ident = pool.tile([P, P], F32)
        nc.gpsimd.dma_start(ident[:], identh)

        # preload the Exp ACT table; reading from the first x chunk makes it
        # wait until that DMA lands, so the table load never delays the ACT
        # ring's own DMA triggers.  Ln loads late, after the last Exp use.
        wz = pool.tile([1, 2], F32)
        warm = pool.tile([1, 2], F32)
        nc.vector.tensor_copy(warm[:], x_sb[0:1, 0:2])
        nc.scalar.activation(wz[:], warm[:], AF.Exp)

        # PE warmup: HAM un-throttles (1.2 -> 2.4 GHz) after ~3.4us of
        # sustained activity.  The explicit dep on the first stream matmul
        # keeps the dummies ORDERED BEFORE the stream in the PE FIFO (the
        # scheduler would otherwise interleave them into it).
        wmm = pool.tile([P, 512], BF16)
        nc.vector.memset(wmm[:], 0.0)
        wms = pool.tile([P, 1], BF16)
        nc.vector.memset(wms[:], 0.0)
        last_dummy = None
        for i in range(6):
            wmo = psum.tile([1, 512], F32, name="wmo", tag="wmo", bufs=2)
            last_dummy = nc.tensor.matmul(wmo[:], wms[:], wmm[:], start=True, stop=True)

        # single accumulation chain in chunk-emission order
        ps4 = psum.tile([3, TB], F32)
        nchunks = len(XCHUNKS)
        first_mm = None
        for ci, (path, blks) in enumerate(XCHUNKS):
            for j, n in enumerate(blks):
                mm = nc.tensor.matmul(
                    ps4[:],
                    m4[:, n, :],
                    x_sb[:, n * TB : (n + 1) * TB],
                    start=(ci == 0 and j == 0),
                    stop=(ci == nchunks - 1 and j == len(blks) - 1),
                )
                if first_mm is None:
                    first_mm = mm
        add_dep_helper(first_mm.ins, last_dummy.ins, sync=False, reason="PE warmup before stream")
        sbl = pool.tile([3, TB], F32)
        nc.vector.tensor_copy(sbl[:], ps4[:])

        # gating, vectorized across all 4 token groups: t16[:, g, :] holds
        # [delta=l0-l1, sd=s0-s1, s1] for tokens g*128..g*128+127 (the
        # differences come straight off the stream via host-prepped
        # [wg0-wg1, v0-v1, v1] stationary columns)
        t16_ps = psum.tile([P, NG, 3], F32)
        for g in range(NG):
            nc.tensor.transpose(
                t16_ps[:, g, :], sbl[0:3, g * P : (g + 1) * P], ident[0:3, 0:3]
            )
        t16 = pool.tile([P, NG, 3], F32)
        nc.vector.tensor_add(t16[:], t16_ps[:], cb16[:])
        ndl = pool.tile([P, NG], F32)
        nc.vector.tensor_sub(ndl[:, :, None], zz[:, :, None], t16[:, :, 0:1])
        mneg = pool.tile([P, NG], F32)
        nc.vector.tensor_tensor(
            mneg[:, :, None], t16[:, :, 0:1], ndl[:, :, None], op=ALU.min
        )  # -|delta|
        eneg = pool.tile([P, NG], F32)
        nc.scalar.activation(eneg[:], mneg[:], AF.Exp)
        den = pool.tile([P, NG], F32)
        nc.vector.tensor_scalar_add(den[:], eneg[:], one1[:])
        gate = pool.tile([P, NG], F32)
        nc.vector.reciprocal(gate[:], den[:])  # = max softmax prob
        mask = pool.tile([P, NG], F32)
        nc.vector.tensor_tensor(
            mask[:, :, None], t16[:, :, 0:1], zz[:, :, None], op=ALU.is_ge
        )
        msd = pool.tile([P, NG], F32)
        nc.vector.tensor_mul(msd[:, :, None], mask[:, :, None], t16[:, :, 1:2])
        ssel = pool.tile([P, NG], F32)
        nc.vector.tensor_add(ssel[:, :, None], msd[:, :, None], t16[:, :, 2:3])
        moe_sb = pool.tile([P, NG], F32)
        nc.vector.tensor_mul(moe_sb[:], gate[:], ssel[:])

        # row log_softmax over all 512 tokens, via PE transposes; the max
        # reduction runs on the [128, 4] layout in parallel with the big
        # transpose (128 lanes instead of 4)
        m128 = pool.tile([P, 1], F32)
        nc.vector.reduce_max(m128[:], moe_sb[:], axis=AX.X)
        tp4 = psum.tile([NG, P], F32)
        nc.tensor.transpose(tp4[:], moe_sb[:], ident[:])
        mtp = psum.tile([1, P], F32, name="mtp", tag="t1", bufs=1)
        nc.tensor.transpose(mtp[:], m128[:], ident[:])
        sb4t = pool.tile([NG, P], F32)
        nc.vector.tensor_copy(sb4t[:], tp4[:])
        negm2 = pool.tile([1, 1], F32)
        nc.vector.reduce_max(negm2[:], mtp[:], axis=AX.X, negate=True)
        negm4 = pool.tile([NG, 1], F32)
        nc.gpsimd.partition_broadcast(negm4[:], negm2[:])
        e4 = pool.tile([NG, P], F32)
        s4 = pool.tile([NG, 1], F32)
        nc.scalar.activation(e4[:], sb4t[:], AF.Exp, bias=negm4[:], accum_out=s4[:])
        s1p = psum.tile([1, NG], F32, name="s1p", tag="t1", bufs=1)
        nc.tensor.transpose(s1p[:], s4[:], ident[0:NG, 0:NG])
        ssum = pool.tile([1, 1], F32)
        nc.vector.reduce_sum(ssum[:], s1p[:], axis=AX.X)
        # log(ssum) via exponent/mantissa bit extraction on the DVE: avoids
        # the Exp->Ln ACT table switch (a 1.28us table load on the critical
        # path).  log(2^k * m) ~= (k + (m-1)) * ln2, max err 0.06 abs --
        # tiny next to the 2e-2 * 175 = 3.5 abs output tolerance.
        I32 = mybir.dt.int32
        si = ssum[0:1, 0:1].bitcast(I32)
        sr = pool.tile([1, 1], I32)
        nc.vector.tensor_scalar(sr[:], si, 23, None, op0=ALU.arith_shift_right)
        kf = pool.tile([1, 1], F32)
        nc.vector.tensor_copy(kf[:], sr[:])
        mi = pool.tile([1, 1], I32)
        nc.vector.tensor_scalar(
            mi[:], si, 0x7FFFFF, 0x3F800000, op0=ALU.bitwise_and, op1=ALU.bitwise_or
        )
        tsum = pool.tile([1, 1], F32)
        nc.vector.tensor_add(tsum[:], kf[:], mi[0:1, 0:1].bitcast(F32))
        logs = pool.tile([1, 1], F32)
        nc.vector.tensor_scalar(
            logs[:], tsum[:], -128.0, 0.6931471805599453, op0=ALU.add, op1=ALU.mult
        )
        shift = pool.tile([1, 1], F32)
        nc.vector.tensor_sub(shift[:], negm2[:], logs[:])
        shift4 = pool.tile([NG, 1], F32)
        nc.gpsimd.partition_broadcast(shift4[:], shift[:])
        res4 = pool.tile([NG, P], F32)
        nc.vector.tensor_scalar_add(res4[:], sb4t[:], shift4[:])
        nc.sync.dma_start(out.rearrange("x (g p) -> g (x p)", p=P), res4[:])


_CACHED = {}


def build_program(which):
    if which in _CACHED:
        return _CACHED[which]
    nc = bacc.Bacc(
        "TRN2",
        target_bir_lowering=False,
        debug=False,
        enable_asserts=False,
        num_devices=NCORES,
    )
    if which == "a":
        io = {
            "w1c": nc.dram_tensor("w1c", [P, E * D], BF16, kind="ExternalInput").ap(),
            "w2c": nc.dram_tensor("w2c", [P, E * D], BF16, kind="ExternalInput").ap(),
            "b1c": nc.dram_tensor("b1c", [P, E], F32, kind="ExternalInput").ap(),
            "b2c": nc.dram_tensor("b2c", [1, E * DC], F32, kind="ExternalInput").ap(),
            "vout": nc.dram_tensor("vout", [P, E * NB], F32, kind="ExternalOutput").ap(),
            "cout": nc.dram_tensor("cout", [1, E], F32, kind="ExternalOutput").ap(),
        }
        emit = emit_phase_a
    else:
        io = {
            "xc": nc.dram_tensor("xc", [P, NB * TB], F32R, kind="ExternalInput").ap(),
            "m4h": nc.dram_tensor("m4h", [P, NB, 3], F32R, kind="ExternalInput").ap(),
            "identh": nc.dram_tensor("identh", [P, P], F32, kind="ExternalInput").ap(),
            "consth": nc.dram_tensor(
                "consth", [P, NG * 3 + NG + 1], F32, kind="ExternalInput"
            ).ap(),
            "out": nc.dram_tensor("out", [1, TB], F32, kind="ExternalOutput").ap(),
        }
        emit = emit_phase_b
    with tile.TileContext(nc) as tc:
        emit(nc, tc, io)
    nc.compile()
    _CACHED[which] = nc
    return nc


def shard_inputs_a(Wg, W1, b1, W2, b2):
    import ml_dtypes

    W1 = np.asarray(W1, np.float32)
    b1 = np.asarray(b1, np.float32)
    W2 = np.asarray(W2, np.float32)
    b2 = np.asarray(b2, np.float32)
    in_maps = []
    for c in range(NCORES):
        hs, he = c * HC, (c + 1) * HC
        # w1c[p, e*D+d] = W1[e, d, hs+p];  w2c[p, e*D+d] = W2[e, hs+p, d]
        in_maps.append(
            {
                "w1c": np.ascontiguousarray(
                    W1[:, :, hs:he].transpose(2, 0, 1).reshape(HC, E * D)
                ).astype(ml_dtypes.bfloat16),
                "w2c": np.ascontiguousarray(
                    W2[:, hs:he, :].transpose(1, 0, 2).reshape(HC, E * D)
                ).astype(ml_dtypes.bfloat16),
                "b1c": np.ascontiguousarray(b1[:, hs:he].T),  # [128, E]
                "b2c": np.ascontiguousarray(
                    b2[:, c * DC : (c + 1) * DC].reshape(1, E * DC)
                ),
            }
        )
    return in_maps


def shard_inputs_b(x, Wg, vpart_sum):
    x = np.asarray(x, np.float32)
    Wg = np.asarray(Wg, np.float32)
    vp = np.asarray(vpart_sum, np.float32).reshape(-1)
    # m4h[p, n, :] = [wg0-wg1, v0-v1, v1] at d = n*128+p
    m4h = np.empty((P, NB, 3), np.float32)
    wg_pn = Wg.reshape(NB, P, E).transpose(1, 0, 2)  # [p, n, e]
    m4h[:, :, 0] = wg_pn[:, :, 0] - wg_pn[:, :, 1]
    v0 = vp[0:D].reshape(NB, P).T
    v1 = vp[D : 2 * D].reshape(NB, P).T
    m4h[:, :, 1] = v0 - v1
    m4h[:, :, 2] = v1
    m4h = np.ascontiguousarray(m4h)
    identh = np.eye(P, dtype=np.float32)
    # consth = [cb (NG*3: [0, c0-c1, c1] per group) | zz (NG) | one (1)]
    consth = np.zeros((P, NG * 3 + NG + 1), np.float32)
    cs = vp[2 * D : 2 * D + E]
    for g in range(NG):
        consth[:, g * 3 + 1] = cs[0] - cs[1]
        consth[:, g * 3 + 2] = cs[1]
    consth[:, NG * 3 + NG] = 1.0
    in_maps = []
    for c in range(NCORES):
        row = c % B
        # xc[p, n*TB + t] = x[row, t, n*128 + p]
        xr = np.ascontiguousarray(
            x[row].reshape(TB, NB, P).transpose(2, 1, 0).reshape(P, NB * TB)
        )
        in_maps.append({"xc": xr, "m4h": m4h, "identh": identh, "consth": consth})
    return in_maps


def run_a(in_maps, **kwargs):
    return bass_utils.run_bass_kernel_spmd(
        build_program("a"), in_maps, core_ids=list(range(NCORES)), **kwargs
    )


def run_b(in_maps, **kwargs):
    return bass_utils.run_bass_kernel_spmd(
        build_program("b"), in_maps, core_ids=list(range(NCORES)), **kwargs
    )


def combine_vparts(res_a):
    """Sum the 8 per-core [128, 32] v partials + [1, 2] c partials into the
    flat [1, 2D+2] vpart layout (v0 | v1 | c0 c1) launch B consumes.

    vout[p, k] = v_partial[k*128//D][(k%NB)*128 + p]."""
    v128 = np.sum([res_a.results[c]["vout"] for c in range(NCORES)], axis=0)
    cs = np.sum([res_a.results[c]["cout"] for c in range(NCORES)], axis=0)
    v = v128.T.reshape(E, NB, P).reshape(E * D)  # [e, j, p] -> flat
    vpart = np.concatenate([v, cs.reshape(-1)]).reshape(1, VPART)
    return np.ascontiguousarray(vpart, np.float32)


def kernel(x, Wg, W1, b1, W2, b2):
    res_a = run_a(shard_inputs_a(Wg, W1, b1, W2, b2))
    # cross-core combine: sum of the 8 per-core partials (the gather/reshard
    # step between the two launches; 16KB, no model math beyond the reduction)
    vpart = combine_vparts(res_a)
    res_b = run_b(shard_inputs_b(x, Wg, vpart))
    return np.concatenate([res_b.results[b]["out"] for b in range(B)], axis=0)
